# revision 20
# baseline (speedup 1.0000x reference)
"""Trainium2 Bass kernel for a dense transformer block (fp8 DoubleRow version).

Reference math (B=32, S=577, D=768, H=12, DH=64, F=3072, fp32):
  h  = LN1(x);  q,k,v = per-head projections of h
  scores = q @ k^T / sqrt(DH)
  probs  = softmax(scores, axis=QUERY)       # quirk: softmax over the query axis
  attn   = probs @ v;  x2 = x + concat(attn) @ Wo + bo
  out    = x2 + (gelu(LN2(x2) @ W1 + b1) @ W2 + b2)

Strategy: pure data-parallel over batch, 4 batch items per core on 8 cores, no
collectives.  On-chip activations live transposed [feature on partitions,
token on free dim].  All large matmuls run in fp8(e4m3) with DoubleRow perf
mode: operands are laid out in (256-row pair, 128-partition) chunks so each
matmul instruction contracts 256 elements, doubling PE throughput vs bf16.
Weights are pre-scaled by 32 on the host (away from fp8 denormals), activations
by 4; scales are folded into the PSUM->SBUF copies and activation-function
scale arguments.  LN stats are computed from fp8 casts of x via DoubleRow
ones-matmuls (sum + sum-of-squares rows in one PSUM tile).  The Pool engine
(idle otherwise) takes the residual adds and the LN2 fp8 casts/squares.
Residual x stays fp32 and resident in SBUF (no DRAM re-read).
"""

import numpy as np
import ml_dtypes

B, S, D, H, DH, F = 32, 577, 768, 12, 64, 3072
NCORES = 8
BPC = B // NCORES          # batches per core
EPS = 1e-5
NCD = D // 128             # 6  d-chunks
NC2 = D // 256             # 3  d-pair-chunks (DoubleRow)
NCF = F // 128             # 24 f-chunks
NF2 = F // 256             # 12 f-pair-chunks
NHP = H // 2               # 6  head pairs
SSPL = [(0, 512), (512, S - 512)]              # free-dim splits of S (psum bank)
DSPL = [(0, 512), (512, D - 512)]              # free-dim splits of D
TCH = [(i * 128, min(128, S - i * 128)) for i in range((S + 127) // 128)]  # 5 t-chunks
SP = 592                   # fp8 pair tiles pad S to 592 (DoubleRow needs pair step % 16 == 0)

AW = 32.0                  # fp8 weight scale
AZ = 4.0                   # fp8 z (normalized act) scale
AQ = AZ * AW / 4.0         # qt/kt store AQ*q (32) to stay inside fp8 range
AV = 512.0                 # vsc = AV * v/rs
ACAT = 4.0                 # concat fp8 scale

_NC_CACHE = {}


def _build_nc(gelu_kind: str = "gelu", bpc: int = BPC):
    from contextlib import ExitStack
    import concourse.bass as bass
    import concourse.tile as tile
    from concourse import bacc, mybir

    f32, bf16 = mybir.dt.float32, mybir.dt.bfloat16
    f8 = mybir.dt.float8e4
    AF = mybir.ActivationFunctionType
    ALU = mybir.AluOpType
    DR = mybir.MatmulPerfMode.DoubleRow
    GELU = {"gelu": AF.Gelu, "tanh": AF.Tanh}[gelu_kind]

    nc = bacc.Bacc("TRN2", target_bir_lowering=False, dynamic_dma_scratch_size=2048)
    xT_d = nc.declare_dram_parameter("xT", [bpc, D, S], f32, isOutput=False)
    x8_d = nc.declare_dram_parameter("x8", [bpc, NC2, 2, 128, S], f8, isOutput=False)
    sq8_d = nc.declare_dram_parameter("sq8", [bpc, NC2, 2, 128, S], f8, isOutput=False)
    wq_d = nc.declare_dram_parameter("wq", [NC2, 2, 128, D], f8, isOutput=False)
    wk_d = nc.declare_dram_parameter("wk", [NC2, 2, 128, D], f8, isOutput=False)
    wv_d = nc.declare_dram_parameter("wv", [NC2, 2, 128, D], f8, isOutput=False)
    wo_d = nc.declare_dram_parameter("wo", [NC2, 2, 128, D], f8, isOutput=False)
    w1_d = nc.declare_dram_parameter("w1", [NC2, 2, 128, F], f8, isOutput=False)
    w2_d = nc.declare_dram_parameter("w2", [NF2, 2, 128, D], f8, isOutput=False)
    bq_d = nc.declare_dram_parameter("bq", [NCD, 128], f32, isOutput=False)
    bk_d = nc.declare_dram_parameter("bk", [NCD, 128], f32, isOutput=False)
    bv_d = nc.declare_dram_parameter("bv", [1, D], bf16, isOutput=False)
    bo_d = nc.declare_dram_parameter("bo", [NCD, 128], f32, isOutput=False)
    wosum_d = nc.declare_dram_parameter("wosum", [NC2, 2, 128, 32], f8, isOutput=False)
    borow_d = nc.declare_dram_parameter("borow", [1, D], bf16, isOutput=False)
    b2row_d = nc.declare_dram_parameter("b2row", [1, D], bf16, isOutput=False)
    b1_d = nc.declare_dram_parameter("b1", [NCF, 128], f32, isOutput=False)
    b2_d = nc.declare_dram_parameter("b2", [NCD, 128], f32, isOutput=False)
    outT_d = nc.declare_dram_parameter("outT", [bpc, D, S], f32, isOutput=True)

    with tile.TileContext(nc) as tc:
        with ExitStack() as ctx:
            wp = ctx.enter_context(tc.tile_pool(name="wp", bufs=1))
            rp = ctx.enter_context(tc.tile_pool(name="rp", bufs=2))      # residual f32
            zp = ctx.enter_context(tc.tile_pool(name="zp", bufs=1))      # fp8 acts
            qkp = ctx.enter_context(tc.tile_pool(name="qkp", bufs=1))    # qt/kt/v/concat
            ep = ctx.enter_context(tc.tile_pool(name="ep", bufs=2))      # exp tiles
            gp = ctx.enter_context(tc.tile_pool(name="gp", bufs=1))      # gelu acts
            sp_ = ctx.enter_context(tc.tile_pool(name="sp", bufs=1))     # small stat rows
            tp = ctx.enter_context(tc.tile_pool(name="tp", bufs=1))      # [128,S] temps
            mmp = ctx.enter_context(tc.tile_pool(name="mmp", bufs=4, space="PSUM"))

            # ---- weights / constants (resident); weight DMAs deferred until
            # after the first x-shard load so compute starts immediately ----
            wq_s = wp.tile([128, NC2, 2, D], f8, name="wq_s")
            wk_s = wp.tile([128, NC2, 2, D], f8, name="wk_s")
            wv_s = wp.tile([128, NC2, 2, D], f8, name="wv_s")
            wo_s = wp.tile([128, NC2, 2, D], f8, name="wo_s")
            w1_s = wp.tile([128, NC2, 2, F], f8, name="w1_s")
            w2_s = wp.tile([128, NF2, 2, D], f8, name="w2_s")

            def emit_load_weights():
                for t_, d_ in ((wq_s, wq_d), (wk_s, wk_d), (wv_s, wv_d),
                               (wo_s, wo_d), (w1_s, w1_d), (w2_s, w2_d)):
                    nc.sync.dma_start(out=t_[:, :, :, :],
                                      in_=d_.ap().rearrange("c j p n -> p c j n"))
            bqs = wp.tile([128, NCD], f32, name="bqs")
            nc.sync.dma_start(out=bqs[:, :], in_=bq_d.ap().rearrange("c p -> p c"))
            bks = wp.tile([128, NCD], f32, name="bks")
            nc.sync.dma_start(out=bks[:, :], in_=bk_d.ap().rearrange("c p -> p c"))
            bvs = wp.tile([1, D], bf16, name="bvs")
            nc.sync.dma_start(out=bvs[:, :], in_=bv_d[:, :])
            bos = wp.tile([128, NCD], f32, name="bos")
            nc.sync.dma_start(out=bos[:, :], in_=bo_d.ap().rearrange("c p -> p c"))
            wosum_s = wp.tile([128, NC2, 2, 32], f8, name="wosum_s")
            nc.sync.dma_start(out=wosum_s[:, :, :, :],
                              in_=wosum_d.ap().rearrange("c j p m -> p c j m"))
            borow = wp.tile([1, D], bf16, name="borow")
            nc.sync.dma_start(out=borow[:, :], in_=borow_d[:, :])
            b2row = wp.tile([1, D], bf16, name="b2row")
            nc.sync.dma_start(out=b2row[:, :], in_=b2row_d[:, :])
            b1s = wp.tile([128, NCF], f32, name="b1s")
            nc.sync.dma_start(out=b1s[:, :], in_=b1_d.ap().rearrange("c p -> p c"))
            b2s = wp.tile([128, NCD], f32, name="b2s")
            nc.sync.dma_start(out=b2s[:, :], in_=b2_d.ap().rearrange("c p -> p c"))
            ones1 = wp.tile([1, 128], bf16, name="ones1")
            nc.vector.memset(ones1[:, :], 1.0)
            # 32-wide ones (pair layout): DoubleRow needs pair step % 16 == 0 and
            # a >=32-partition matmul destination
            ones8 = wp.tile([128, 2, 32], f8, name="ones8")
            nc.vector.memset(ones8[:, :, :], 1.0)
            eps_s = wp.tile([1, 1], f32, name="eps_s")
            nc.vector.memset(eps_s[:, :], EPS)
            onesS = wp.tile([1, S], bf16, name="onesS")
            nc.vector.memset(onesS[:, :], 1.0)
            ones128f = wp.tile([128, 1], f32, name="ones128f")
            nc.vector.memset(ones128f[:, :], 1.0)
            # bias-of-v broadcast tile [128, D] bf16 (built once via rank-1 matmul)
            bvt = wp.tile([128, D], bf16, name="bvt")
            bvp = mmp.tile([128, D], f32, name="bvp", tag="mm", padded_shape=[128, 1024])
            for (s0, sn) in DSPL:
                nc.tensor.matmul(bvp[:, s0:s0 + sn], ones1[0:1, :], bvs[0:1, s0:s0 + sn],
                                 start=True, stop=True)
            nc.vector.tensor_scalar_mul(bvt[:, :], bvp[:, 0:D], 1.0)
            # scalar sum(bo)/D for the LN2 mean (runtime value)
            bop = mmp.tile([128, 64], f32, name="bop", tag="mm", padded_shape=[128, 1024])
            nc.tensor.matmul(bop[0:1, 0:NCD], ones128f[:, :], bos[:, :],
                             start=True, stop=True)
            bosum = wp.tile([1, 1], f32, name="bosum")
            nc.vector.tensor_reduce(bosum[0:1, 0:1], bop[0:1, 0:NCD],
                                    mybir.AxisListType.X, ALU.add)
            nc.vector.tensor_scalar_mul(bosum[0:1, 0:1], bosum[0:1, 0:1], 1.0 / D)

            # ---------------- helpers ----------------
            def emit_stats(x8t, sq8t, spt, sqt, c2, first, last):
                """DoubleRow ones-matmul accumulation of per-column sums (spt)
                and sums of squares (sqt), rows 0-31 of separate base-0 PSUM
                tiles (DoubleRow requires dst base partition 0).  DoubleRow for
                the 512-wide split only (FD>=128); plain fp8 for the 65 tail."""
                for (s0, sn) in SSPL:
                    if sn >= 128:
                        nc.tensor.matmul(spt[0:32, s0:s0 + sn], ones8[:, :, :],
                                         x8t[:, c2, :, s0:s0 + sn],
                                         start=first, stop=last, perf_mode=DR)
                        nc.tensor.matmul(sqt[0:32, s0:s0 + sn], ones8[:, :, :],
                                         sq8t[:, c2, :, s0:s0 + sn],
                                         start=first, stop=last, perf_mode=DR)
                    else:
                        for j in range(2):
                            nc.tensor.matmul(spt[0:32, s0:s0 + sn], ones8[:, j, :],
                                             x8t[:, c2, j, s0:s0 + sn],
                                             start=first and j == 0,
                                             stop=last and j == 1)
                            nc.tensor.matmul(sqt[0:32, s0:s0 + sn], ones8[:, j, :],
                                             sq8t[:, c2, j, s0:s0 + sn],
                                             start=first and j == 0,
                                             stop=last and j == 1)

            def emit_chain(spt, sqt, mu_bias=None):
                """LN scalar chain on [1,S] rows; outputs AZ*rstd and -AZ*mu*rstd."""
                mu_s = sp_.tile([1, S], f32, name="mu_s", tag="mu_s", bufs=2)
                if mu_bias is None:
                    nc.vector.tensor_scalar_mul(mu_s[:, :], spt[0:1, :], 1.0 / D)
                else:
                    nc.vector.tensor_scalar(mu_s[:, :], spt[0:1, :], 1.0 / D,
                                            mu_bias, op0=ALU.mult, op1=ALU.add)
                v_s = sp_.tile([1, S], f32, name="v_s", tag="v_s", bufs=2)
                nc.scalar.activation(v_s[:, :], sqt[0:1, :], AF.Copy, scale=1.0 / D)
                nc.vector.tensor_mul(spt[0:1, :], mu_s[:, :], mu_s[:, :])
                nc.vector.tensor_sub(v_s[:, :], v_s[:, :], spt[0:1, :])
                w_s = tp.tile([1, S], f32, name="w_s", tag="w_s", bufs=2)
                nc.scalar.activation(w_s[:, :], v_s[:, :], AF.Sqrt, bias=eps_s[0:1, 0:1])
                nc.vector.reciprocal_approx_fast(v_s[:, :], w_s[:, :])
                rstd_bf = sp_.tile([1, S], bf16, name="rstd_bf", tag="rstdbf", bufs=2)
                nc.scalar.activation(rstd_bf[:, :], v_s[:, :], AF.Copy, scale=AZ)
                nmr_bf = sp_.tile([1, S], bf16, name="nmr_bf", tag="nmrbf", bufs=2)
                nc.vector.scalar_tensor_tensor(nmr_bf[:, :], mu_s[:, :], -AZ, v_s[:, :],
                                               op0=ALU.mult, op1=ALU.mult)
                return rstd_bf, nmr_bf

            def emit_bcast(row_bf):
                """Broadcast a [1,S] bf16 row across 128 partitions via rank-1 matmul."""
                bc = mmp.tile([128, S], f32, name="bc", tag="mm", padded_shape=[128, 1024])
                for (s0, sn) in SSPL:
                    nc.tensor.matmul(bc[:, s0:s0 + sn], ones1[0:1, :], row_bf[0:1, s0:s0 + sn],
                                     start=True, stop=True)
                return bc

            def emit_znorm(src, rstd_bc, nmr_bc, z):
                """z[:,c2,j,:] = fp8(src[:,c,:] * (AZ*rstd) + (-AZ*mu*rstd))."""
                for c in range(NCD):
                    c2, j = divmod(c, 2)
                    zt = tp.tile([128, S], bf16, name="zt", tag="zt", bufs=2)
                    nc.vector.tensor_mul(zt[:, :], src[:, c, :], rstd_bc[:, 0:S])
                    nc.vector.tensor_add(z[:, c2, j, 0:S], zt[:, :], nmr_bc[:, 0:S])

            # ---------------- phase emitters ----------------
            xt_tiles = [None] * bpc
            x8_tiles = [None] * bpc
            sq8_tiles = [None] * bpc
            z1_tiles = [None] * bpc
            z2_tiles = [None] * bpc
            x2_tiles = [None] * bpc
            ch1 = [None] * bpc
            ch2 = [None] * bpc

            def emit_load_x(b):
                x8t = zp.tile([128, NC2, 2, SP], f8, name="x8t", tag="x8t", bufs=2)
                nc.sync.dma_start(out=x8t[:, :, :, 0:S],
                                  in_=x8_d[b].rearrange("c j p s -> p c j s"))
                sq8t = zp.tile([128, NC2, 2, SP], f8, name="sq8t", tag="sq8t", bufs=2)
                nc.sync.dma_start(out=sq8t[:, :, :, 0:S],
                                  in_=sq8_d[b].rearrange("c j p s -> p c j s"))
                xt = rp.tile([128, NCD, S], f32, name="xt", tag="xt", bufs=2)
                for c in range(NCD):
                    nc.sync.dma_start(out=xt[:, c, :],
                                      in_=xT_d[b, c * 128:(c + 1) * 128, :])
                x8_tiles[b], sq8_tiles[b], xt_tiles[b] = x8t, sq8t, xt

            def emit_stats1(b):
                spt = mmp.tile([128, S], f32, name="spt", tag="mm", padded_shape=[128, 1024])
                sqt = mmp.tile([128, S], f32, name="sqt", tag="mm", padded_shape=[128, 1024])
                for c2 in range(NC2):
                    emit_stats(x8_tiles[b], sq8_tiles[b], spt, sqt, c2,
                               c2 == 0, c2 == NC2 - 1)
                ch1[b] = emit_chain(spt, sqt)

            def emit_zfinish1(b):
                rstd_bf, nmr_bf = ch1[b]
                rbc = emit_bcast(rstd_bf)
                nbc = emit_bcast(nmr_bf)
                z1 = zp.tile([128, NC2, 2, SP], f8, name="z1", tag="z1", bufs=2)
                emit_znorm(xt_tiles[b], rbc, nbc, z1)
                z1_tiles[b] = z1

            def emit_attention(b, interleave=()):
                """interleave: callables emitted between head-pairs to feed the
                PE while ACT grinds the softmax exps."""
                interleave = list(interleave)
                z1 = z1_tiles[b]
                # --- QKV projections (DoubleRow fp8) ---
                qt = qkp.tile([128, NHP, S], f8, name="qt", tag="qt")
                kt = qkp.tile([128, NHP, S], f8, name="kt", tag="kt")
                for hp in range(NHP):
                    hc = slice(hp * 128, (hp + 1) * 128)
                    qps = mmp.tile([128, S], f32, name="qps", tag="mm", padded_shape=[128, 1024])
                    for (s0, sn) in SSPL:
                        if sn >= 128:
                            for c2 in range(NC2):
                                nc.tensor.matmul(qps[:, s0:s0 + sn], wq_s[:, c2, :, hc],
                                                 z1[:, c2, :, s0:s0 + sn],
                                                 start=(c2 == 0), stop=(c2 == NC2 - 1),
                                                 perf_mode=DR)
                        else:
                            for c in range(NCD):
                                c2, j = divmod(c, 2)
                                nc.tensor.matmul(qps[:, s0:s0 + sn], wq_s[:, c2, j, hc],
                                                 z1[:, c2, j, s0:s0 + sn],
                                                 start=(c == 0), stop=(c == NCD - 1))
                    nc.vector.tensor_scalar(qt[:, hp, :], qps[:, 0:S],
                                            AQ / (AZ * AW), bqs[:, hp:hp + 1],
                                            op0=ALU.mult, op1=ALU.add)
                    kps = mmp.tile([128, S], f32, name="kps", tag="mm", padded_shape=[128, 1024])
                    for (s0, sn) in SSPL:
                        if sn >= 128:
                            for c2 in range(NC2):
                                nc.tensor.matmul(kps[:, s0:s0 + sn], wk_s[:, c2, :, hc],
                                                 z1[:, c2, :, s0:s0 + sn],
                                                 start=(c2 == 0), stop=(c2 == NC2 - 1),
                                                 perf_mode=DR)
                        else:
                            for c in range(NCD):
                                c2, j = divmod(c, 2)
                                nc.tensor.matmul(kps[:, s0:s0 + sn], wk_s[:, c2, j, hc],
                                                 z1[:, c2, j, s0:s0 + sn],
                                                 start=(c == 0), stop=(c == NCD - 1))
                    nc.vector.tensor_scalar(kt[:, hp, :], kps[:, 0:S],
                                            AQ / (AZ * AW), bks[:, hp:hp + 1],
                                            op0=ALU.mult, op1=ALU.add)
                # V in natural layout [t, v]: z1 chunks as stationary, wv moving
                v = qkp.tile([128, len(TCH), D], bf16, name="v", tag="v")
                vsc = qkp.tile([128, len(TCH), D], f8, name="vsc", tag="vsc")
                for it, (t0, tw) in enumerate(TCH):
                    vps = mmp.tile([128, D], f32, name="vps", tag="mm", padded_shape=[128, 1024])
                    for (s0, sn) in DSPL:
                        for c2 in range(NC2):
                            nc.tensor.matmul(vps[0:tw, s0:s0 + sn],
                                             z1[:, c2, :, t0:t0 + tw],
                                             wv_s[:, c2, :, s0:s0 + sn],
                                             start=(c2 == 0), stop=(c2 == NC2 - 1),
                                             perf_mode=DR)
                    nc.vector.tensor_add(v[0:tw, it, :], vps[0:tw, 0:D], bvt[0:tw, :])

                # --- per-head-pair attention ---
                concat = qkp.tile([128, NC2, 2, SP], f8, name="concat", tag="concat")
                for hp in range(NHP):
                    etiles = [None, None]
                    rstiles = [None, None]
                    rectiles = [None, None]
                    for h2 in range(2):
                        hb = h2 * 64
                        e = ep.tile([128, len(TCH), SP], f8, name="e", tag="e")
                        rs = sp_.tile([128, len(TCH)], f32, name="rs", tag="rs", bufs=2)
                        rec = sp_.tile([128, len(TCH)], f32, name="rec", tag="rec", bufs=2)
                        hcol = slice(hp * 128 + hb, hp * 128 + hb + 64)
                        for it, (t0, tw) in enumerate(TCH):
                            stps = mmp.tile([128, S], f32, name="stps", tag="mm",
                                            padded_shape=[128, 1024])
                            for (s0, sn) in SSPL:
                                nc.tensor.matmul(stps[0:tw, s0:s0 + sn],
                                                 kt[hb:hb + 64, hp, t0:t0 + tw],
                                                 qt[hb:hb + 64, hp, s0:s0 + sn],
                                                 start=True, stop=True)
                            nc.scalar.activation(e[0:tw, it, 0:S], stps[0:tw, 0:S], AF.Exp,
                                                 bias=0.0,
                                                 scale=float(1.0 / (np.sqrt(DH) * AQ * AQ)),
                                                 accum_out=rs[0:tw, it:it + 1])
                            # per-chunk reciprocal + v scaling right behind each
                            # exp: vsc(it) is ready ~2 exps after its accum
                            nc.vector.reciprocal(rec[0:tw, it:it + 1],
                                                 rs[0:tw, it:it + 1])
                            nc.vector.tensor_scalar(vsc[0:tw, it, hcol],
                                                    v[0:tw, it, hcol],
                                                    rec[0:tw, it:it + 1],
                                                    AV / (AZ * AW),
                                                    op0=ALU.mult, op1=ALU.mult)
                        etiles[h2] = e
                        rstiles[h2] = rs
                    # PE work between the exps and attnV so the rec/vsc DVE
                    # latency never stalls the attnV weight loads
                    if interleave:
                        interleave.pop(0)()
                    ap_tiles = [mmp.tile([128, S], f32, name="ap_ps", tag="mm",
                                         padded_shape=[128, 1024]) for _ in range(2)]
                    for h2 in range(2):
                        hb = h2 * 64
                        ap_ps = ap_tiles[h2]
                        hcol = slice(hp * 128 + hb, hp * 128 + hb + 64)
                        e = etiles[h2]
                        for (s0, sn) in SSPL:
                            if sn >= 128:
                                nc.tensor.matmul(ap_ps[0:64, s0:s0 + sn],
                                                 vsc[0:128, 0:2, hcol],
                                                 e[0:128, 0:2, s0:s0 + sn],
                                                 start=True, stop=False, perf_mode=DR)
                                nc.tensor.matmul(ap_ps[0:64, s0:s0 + sn],
                                                 vsc[0:128, 2:4, hcol],
                                                 e[0:128, 2:4, s0:s0 + sn],
                                                 start=False, stop=False, perf_mode=DR)
                                lt0, ltw = TCH[-1]
                                nc.tensor.matmul(ap_ps[0:64, s0:s0 + sn],
                                                 vsc[0:ltw, 4, hcol],
                                                 e[0:ltw, 4, s0:s0 + sn],
                                                 start=False, stop=True)
                            else:
                                for it, (t0, tw) in enumerate(TCH):
                                    nc.tensor.matmul(ap_ps[0:64, s0:s0 + sn],
                                                     vsc[0:tw, it, hcol],
                                                     e[0:tw, it, s0:s0 + sn],
                                                     start=(it == 0),
                                                     stop=(it == len(TCH) - 1))
                    c2h, jh = divmod(hp, 2)
                    nc.vector.tensor_scalar_mul(concat[0:64, c2h, jh, 0:S],
                                                ap_tiles[0][0:64, 0:S], ACAT / AV)
                    nc.vector.tensor_scalar_mul(concat[64:128, c2h, jh, 0:S],
                                                ap_tiles[1][0:64, 0:S], ACAT / AV)

                # --- output projection + residual + fused LN2 stats.
                # Sums: sum_d(x2) = sum_d(x) [from x8] + colsum(Wo).concat + sum(bo)
                # (the bo constant enters via the LN2 chain mu bias).
                # Sumsq: ACT Square of each x2 chunk (fp8) + plain fp8 matmuls. ---
                x2 = rp.tile([128, NCD, S], f32, name="x2", tag="x2", bufs=2)
                sq28 = zp.tile([128, NCD, SP], f8, name="sq28", tag="sq28", bufs=1)
                spt = mmp.tile([128, S], f32, name="spt2", tag="mm", padded_shape=[128, 1024])
                sqt = mmp.tile([128, S], f32, name="sqt2", tag="mm", padded_shape=[128, 1024])
                xt = xt_tiles[b]
                x8t = x8_tiles[b]
                for (s0, sn) in SSPL:
                    for c2 in range(NC2):
                        if sn >= 128:
                            nc.tensor.matmul(spt[0:32, s0:s0 + sn], ones8[:, :, :],
                                             x8t[:, c2, :, s0:s0 + sn],
                                             start=(c2 == 0), stop=False, perf_mode=DR)
                            nc.tensor.matmul(spt[0:32, s0:s0 + sn], wosum_s[:, c2, :, :],
                                             concat[:, c2, :, s0:s0 + sn],
                                             start=False, stop=(c2 == NC2 - 1),
                                             perf_mode=DR)
                        else:
                            for j in range(2):
                                nc.tensor.matmul(spt[0:32, s0:s0 + sn], ones8[:, j, :],
                                                 x8t[:, c2, j, s0:s0 + sn],
                                                 start=(c2 == 0 and j == 0), stop=False)
                                nc.tensor.matmul(spt[0:32, s0:s0 + sn],
                                                 wosum_s[:, c2, j, :],
                                                 concat[:, c2, j, s0:s0 + sn],
                                                 start=False,
                                                 stop=(c2 == NC2 - 1 and j == 1))
                for ec in range(NCD):
                    wops = mmp.tile([128, S], f32, name="wops", tag="mm",
                                    padded_shape=[128, 1024])
                    for (s0, sn) in SSPL:
                        if sn >= 128:
                            for cc in range(NC2):
                                nc.tensor.matmul(wops[:, s0:s0 + sn],
                                                 wo_s[:, cc, :, ec * 128:(ec + 1) * 128],
                                                 concat[:, cc, :, s0:s0 + sn],
                                                 start=(cc == 0), stop=False,
                                                 perf_mode=DR)
                        else:
                            for cc in range(NCD):
                                c2c, jc = divmod(cc, 2)
                                nc.tensor.matmul(wops[:, s0:s0 + sn],
                                                 wo_s[:, c2c, jc, ec * 128:(ec + 1) * 128],
                                                 concat[:, c2c, jc, s0:s0 + sn],
                                                 start=(cc == 0), stop=False)
                        # bias bo via bf16 rank-1 (keeps the SBUF copy a single STT op)
                        nc.tensor.matmul(wops[:, s0:s0 + sn],
                                         borow[0:1, ec * 128:(ec + 1) * 128],
                                         onesS[0:1, s0:s0 + sn],
                                         start=False, stop=True)
                    nc.vector.scalar_tensor_tensor(x2[:, ec, :], wops[:, 0:S],
                                                   1.0 / (ACAT * AW), xt[:, ec, :],
                                                   op0=ALU.mult, op1=ALU.add)
                    nc.vector.tensor_mul(sq28[:, ec, 0:S], x2[:, ec, :], x2[:, ec, :])
                    for (s0, sn) in SSPL:
                        nc.tensor.matmul(sqt[0:32, s0:s0 + sn], ones8[:, 0, :],
                                         sq28[:, ec, s0:s0 + sn],
                                         start=(ec == 0), stop=(ec == NCD - 1))
                x2_tiles[b] = x2
                return spt, sqt

            def emit_stats2(b, spts):
                ch2[b] = emit_chain(*spts, mu_bias=bosum[0:1, 0:1])

            def emit_zfinish2(b):
                rstd_bf, nmr_bf = ch2[b]
                rbc = emit_bcast(rstd_bf)
                nbc = emit_bcast(nmr_bf)
                z2 = zp.tile([128, NC2, 2, SP], f8, name="z2", tag="z2", bufs=1)
                emit_znorm(x2_tiles[b], rbc, nbc, z2)
                z2_tiles[b] = z2

            def emit_fc1(b, g, lo, hi):
                z2 = z2_tiles[b]
                for fc in range(lo, hi):
                    fps = mmp.tile([128, S], f32, name="fps", tag="mm",
                                   padded_shape=[128, 1024])
                    for (s0, sn) in SSPL:
                        if sn >= 128:
                            for c2 in range(NC2):
                                nc.tensor.matmul(fps[:, s0:s0 + sn],
                                                 w1_s[:, c2, :, fc * 128:(fc + 1) * 128],
                                                 z2[:, c2, :, s0:s0 + sn],
                                                 start=(c2 == 0), stop=(c2 == NC2 - 1),
                                                 perf_mode=DR)
                        else:
                            for c in range(NCD):
                                c2, j = divmod(c, 2)
                                nc.tensor.matmul(fps[:, s0:s0 + sn],
                                                 w1_s[:, c2, j, fc * 128:(fc + 1) * 128],
                                                 z2[:, c2, j, s0:s0 + sn],
                                                 start=(c == 0), stop=(c == NCD - 1))
                    cf, jf = divmod(fc, 2)
                    nc.scalar.activation(g[:, cf, jf, 0:S], fps[:, 0:S], GELU,
                                         bias=b1s[:, fc:fc + 1], scale=1.0 / (AZ * AW))

            def emit_fc2_chunk(b, g, ec):
                x2 = x2_tiles[b]
                p2 = mmp.tile([128, S], f32, name="p2", tag="mm", padded_shape=[128, 1024])
                for (s0, sn) in SSPL:
                    if sn >= 128:
                        for cf in range(NF2):
                            nc.tensor.matmul(p2[:, s0:s0 + sn],
                                             w2_s[:, cf, :, ec * 128:(ec + 1) * 128],
                                             g[:, cf, :, s0:s0 + sn],
                                             start=(cf == 0), stop=False,
                                             perf_mode=DR)
                    else:
                        for cf in range(NCF):
                            c2f, jf = divmod(cf, 2)
                            nc.tensor.matmul(p2[:, s0:s0 + sn],
                                             w2_s[:, c2f, jf, ec * 128:(ec + 1) * 128],
                                             g[:, c2f, jf, s0:s0 + sn],
                                             start=(cf == 0), stop=False)
                    nc.tensor.matmul(p2[:, s0:s0 + sn],
                                     b2row[0:1, ec * 128:(ec + 1) * 128],
                                     onesS[0:1, s0:s0 + sn],
                                     start=False, stop=True)
                o1 = tp.tile([128, S], f32, name="o1", tag="o1", bufs=2)
                nc.vector.scalar_tensor_tensor(o1[:, :], p2[:, 0:S], 1.0 / AW,
                                               x2[:, ec, :],
                                               op0=ALU.mult, op1=ALU.add)
                nc.sync.dma_start(out=outT_d[b, ec * 128:(ec + 1) * 128, :],
                                  in_=o1[:, :])

            # ---------------- emission schedule ----------------
            # Two-deep pipeline: FC2(b) is emitted after attention(b+1) so
            # batch b+1's LN2 chain hides under FC2(b)'s matmuls; chain1(b+1)
            # hides under FC1(b); LN2 stats matmuls are fused into Wo.
            emit_load_x(0)
            emit_load_weights()
            emit_stats1(0)          # chain1(0) — exposed at startup only
            emit_zfinish1(0)
            if bpc > 1:
                emit_load_x(1)      # x(b+1) DMAs hide under attention(b)
            il0 = [(lambda: emit_stats1(1))] if bpc > 1 else []
            spt2 = emit_attention(0, interleave=il0)
            emit_stats2(0, spt2)    # chain2(0) — exposed once (no prior FC2)
            for b in range(bpc):
                emit_zfinish2(b)
                g = gp.tile([128, NF2, 2, SP], f8, name="g", tag="g")
                emit_fc1(b, g, 0, 3)
                if b + 1 < bpc:
                    emit_zfinish1(b + 1)    # z1(b+1): chain1 ran during
                                            # attention(b); znorm1 DVE clears
                                            # well before attention(b+1)
                emit_fc1(b, g, 3, NCF)
                if b + 1 < bpc:
                    if b + 2 < bpc:
                        emit_load_x(b + 2)  # prefetch two stages ahead
                    # Interleave into attention(b+1): FC2(b) chunks feed the PE
                    # between head-pairs (ACT-bound); stats1(b+2) + chain1(b+2)
                    # run mid-attention so the LN1 chain is never on the
                    # critical path.
                    il = [(lambda ec=ec: emit_fc2_chunk(b, g, ec)) for ec in range(2)]
                    if b + 2 < bpc:
                        il.append(lambda: emit_stats1(b + 2))
                    il += [(lambda ec=ec: emit_fc2_chunk(b, g, ec)) for ec in range(2, 5)]
                    spt2 = emit_attention(b + 1, interleave=il)
                    emit_stats2(b + 1, spt2)
                    emit_fc2_chunk(b, g, 5)
                else:
                    for ec in range(NCD):
                        emit_fc2_chunk(b, g, ec)
    nc.finalize()
    return nc


def _get_nc(gelu_kind: str = "gelu", bpc: int = BPC):
    key = (gelu_kind, bpc)
    if key not in _NC_CACHE:
        _NC_CACHE[key] = _build_nc(gelu_kind, bpc)
    return _NC_CACHE[key]


def _pair_rows(w):
    """[D_in, N] -> [NC_in/256, 2, 128, N] fp8 pair-chunk layout."""
    d_in, n = w.shape
    return np.ascontiguousarray(w.reshape(d_in // 256, 2, 128, n))


def _prep_inputs(inputs):
    """Host-side prep: fp8 weight quantization (scaled by AW), pair layouts,
    folded LN affines, fp8 x / x^2 stats inputs.  Returns per-core input maps."""
    f8 = ml_dtypes.float8_e4m3
    bf16 = ml_dtypes.bfloat16
    f32 = np.float32
    g1 = np.asarray(inputs["ln1_g"], f32)
    b1l = np.asarray(inputs["ln1_b"], f32)
    g2 = np.asarray(inputs["ln2_g"], f32)
    b2l = np.asarray(inputs["ln2_b"], f32)

    def flat(Wx):  # [H, D, DH] -> [D, H*DH]
        return np.ascontiguousarray(np.transpose(np.asarray(Wx, f32), (1, 0, 2)).reshape(D, D))

    wq_f, wk_f, wv_f = flat(inputs["Wq"]), flat(inputs["Wk"]), flat(inputs["Wv"])
    W1 = np.asarray(inputs["W1"], f32)
    W2 = np.asarray(inputs["W2"], f32)
    Wo = np.asarray(inputs["Wo"], f32)
    bo = np.asarray(inputs["bo"], f32)
    b2b = np.asarray(inputs["b2"], f32)
    # column sums of the *quantized* Wo (for the analytic LN2 mean), scaled 1/ACAT
    wo_q = (AW * Wo).astype(f8).astype(f32) / AW
    wosum = (wo_q.sum(axis=1) / ACAT).astype(f8)          # [D]
    wosum_rep = np.broadcast_to(
        wosum.reshape(NC2, 2, 128, 1), (NC2, 2, 128, 32))
    w = {
        "wq": _pair_rows((AW * g1[:, None] * wq_f).astype(f8)),
        "wk": _pair_rows((AW * g1[:, None] * wk_f).astype(f8)),
        "wv": _pair_rows((AW * g1[:, None] * wv_f).astype(f8)),
        "wo": _pair_rows((AW * Wo).astype(f8)),
        "w1": _pair_rows((AW * g2[:, None] * W1).astype(f8)),
        "w2": _pair_rows((AW * W2).astype(f8)),
        # qt = psum*(AQ/(AZ*AW)) + AQ*bq_fold
        "bq": (AQ * (b1l @ wq_f + np.asarray(inputs["bq"], f32).reshape(-1))).reshape(NCD, 128).astype(f32),
        "bk": (AQ * (b1l @ wk_f + np.asarray(inputs["bk"], f32).reshape(-1))).reshape(NCD, 128).astype(f32),
        # v_sb = psum + (AZ*AW)*bv_fold
        "bv": (AZ * AW * (b1l @ wv_f + np.asarray(inputs["bv"], f32).reshape(-1))).reshape(1, D).astype(bf16),
        "bo": bo.reshape(NCD, 128).copy(),
        "b1": (b2l @ W1 + np.asarray(inputs["b1"], f32)).reshape(NCF, 128).astype(f32),
        "b2": b2b.reshape(NCD, 128).copy(),
        "wosum": np.ascontiguousarray(wosum_rep),
        "borow": (ACAT * AW * bo).reshape(1, D).astype(bf16),
        "b2row": (AW * b2b).reshape(1, D).astype(bf16),
    }
    x = np.asarray(inputs["x"], np.float32)
    # shard over batch, transpose to [b, D, S] per core
    xT = np.ascontiguousarray(
        x.reshape(NCORES, BPC, S, D).swapaxes(2, 3))  # [8, BPC, D, S]
    x8 = xT.astype(f8)                                # [8, BPC, D, S] fp8
    x8f = x8.astype(f32)
    sq8 = (x8f * x8f).astype(f8)
    x8 = x8.reshape(NCORES, BPC, NC2, 2, 128, S)
    sq8 = sq8.reshape(NCORES, BPC, NC2, 2, 128, S)
    return [dict(w, xT=xT[i], x8=x8[i], sq8=sq8[i]) for i in range(NCORES)]


def kernel(**inputs) -> np.ndarray:
    from concourse.bass_utils import run_bass_kernel_spmd

    nc = _get_nc("gelu", BPC)
    in_maps = _prep_inputs(inputs)
    res = run_bass_kernel_spmd(nc, in_maps, core_ids=list(range(NCORES)))
    outs = [res.results[i]["outT"] for i in range(NCORES)]   # each [BPC, D, S]
    out = np.stack(outs, 0).swapaxes(2, 3).reshape(B, S, D)
    return np.ascontiguousarray(out.astype(np.float32))


# revision 21
# speedup vs baseline: 1.0187x; 1.0187x over previous
"""Trainium2 Bass kernel for a dense transformer block (fp8 DoubleRow version).

Reference math (B=32, S=577, D=768, H=12, DH=64, F=3072, fp32):
  h  = LN1(x);  q,k,v = per-head projections of h
  scores = q @ k^T / sqrt(DH)
  probs  = softmax(scores, axis=QUERY)       # quirk: softmax over the query axis
  attn   = probs @ v;  x2 = x + concat(attn) @ Wo + bo
  out    = x2 + (gelu(LN2(x2) @ W1 + b1) @ W2 + b2)

Strategy: pure data-parallel over batch, 4 batch items per core on 8 cores, no
collectives.  On-chip activations live transposed [feature on partitions,
token on free dim].  All large matmuls run in fp8(e4m3) with DoubleRow perf
mode: operands are laid out in (256-row pair, 128-partition) chunks so each
matmul instruction contracts 256 elements, doubling PE throughput vs bf16.
Weights are pre-scaled by 32 on the host (away from fp8 denormals), activations
by 4; scales are folded into the PSUM->SBUF copies and activation-function
scale arguments.  LN stats are computed from fp8 casts of x via DoubleRow
ones-matmuls (sum + sum-of-squares rows in one PSUM tile).  The Pool engine
(idle otherwise) takes the residual adds and the LN2 fp8 casts/squares.
Residual x stays fp32 and resident in SBUF (no DRAM re-read).
"""

import numpy as np
import ml_dtypes

B, S, D, H, DH, F = 32, 577, 768, 12, 64, 3072
NCORES = 8
BPC = B // NCORES          # batches per core
EPS = 1e-5
NCD = D // 128             # 6  d-chunks
NC2 = D // 256             # 3  d-pair-chunks (DoubleRow)
NCF = F // 128             # 24 f-chunks
NF2 = F // 256             # 12 f-pair-chunks
NHP = H // 2               # 6  head pairs
SSPL = [(0, 512), (512, S - 512)]              # free-dim splits of S (psum bank)
DSPL = [(0, 512), (512, D - 512)]              # free-dim splits of D
TCH = [(i * 128, min(128, S - i * 128)) for i in range((S + 127) // 128)]  # 5 t-chunks
SP = 592                   # fp8 pair tiles pad S to 592 (DoubleRow needs pair step % 16 == 0)

AW = 32.0                  # fp8 weight scale
AZ = 4.0                   # fp8 z (normalized act) scale
AQ = AZ * AW / 4.0         # qt/kt store AQ*q (32) to stay inside fp8 range
AV = 512.0                 # vsc = AV * v/rs
ACAT = 4.0                 # concat fp8 scale

_NC_CACHE = {}


def _build_nc(gelu_kind: str = "gelu", bpc: int = BPC):
    from contextlib import ExitStack
    import concourse.bass as bass
    import concourse.tile as tile
    from concourse import bacc, mybir

    f32, bf16 = mybir.dt.float32, mybir.dt.bfloat16
    f8 = mybir.dt.float8e4
    AF = mybir.ActivationFunctionType
    ALU = mybir.AluOpType
    DR = mybir.MatmulPerfMode.DoubleRow
    GELU = {"gelu": AF.Gelu, "tanh": AF.Tanh}[gelu_kind]

    nc = bacc.Bacc("TRN2", target_bir_lowering=False, dynamic_dma_scratch_size=2048)
    xT_d = nc.declare_dram_parameter("xT", [bpc, D, S], f32, isOutput=False)
    x8_d = nc.declare_dram_parameter("x8", [bpc, NC2, 2, 128, S], f8, isOutput=False)
    sq8_d = nc.declare_dram_parameter("sq8", [bpc, NC2, 2, 128, S], f8, isOutput=False)
    wq_d = nc.declare_dram_parameter("wq", [NC2, 2, 128, D], f8, isOutput=False)
    wk_d = nc.declare_dram_parameter("wk", [NC2, 2, 128, D], f8, isOutput=False)
    wv_d = nc.declare_dram_parameter("wv", [NC2, 2, 128, D], f8, isOutput=False)
    wo_d = nc.declare_dram_parameter("wo", [NC2, 2, 128, D], f8, isOutput=False)
    w1_d = nc.declare_dram_parameter("w1", [NC2, 2, 128, F], f8, isOutput=False)
    w2_d = nc.declare_dram_parameter("w2", [NF2, 2, 128, D], f8, isOutput=False)
    bq_d = nc.declare_dram_parameter("bq", [NCD, 128], f32, isOutput=False)
    bk_d = nc.declare_dram_parameter("bk", [NCD, 128], f32, isOutput=False)
    bv_d = nc.declare_dram_parameter("bv", [1, D], bf16, isOutput=False)
    bo_d = nc.declare_dram_parameter("bo", [NCD, 128], f32, isOutput=False)
    wosum_d = nc.declare_dram_parameter("wosum", [NC2, 2, 128, 32], f8, isOutput=False)
    borow_d = nc.declare_dram_parameter("borow", [1, D], bf16, isOutput=False)
    b2row_d = nc.declare_dram_parameter("b2row", [1, D], bf16, isOutput=False)
    b1_d = nc.declare_dram_parameter("b1", [NCF, 128], f32, isOutput=False)
    b2_d = nc.declare_dram_parameter("b2", [NCD, 128], f32, isOutput=False)
    outT_d = nc.declare_dram_parameter("outT", [bpc, D, S], f32, isOutput=True)

    with tile.TileContext(nc) as tc:
        with ExitStack() as ctx:
            wp = ctx.enter_context(tc.tile_pool(name="wp", bufs=1))
            rp = ctx.enter_context(tc.tile_pool(name="rp", bufs=2))      # residual f32
            zp = ctx.enter_context(tc.tile_pool(name="zp", bufs=1))      # fp8 acts
            qkp = ctx.enter_context(tc.tile_pool(name="qkp", bufs=1))    # qt/kt/v/concat
            ep = ctx.enter_context(tc.tile_pool(name="ep", bufs=2))      # exp tiles
            gp = ctx.enter_context(tc.tile_pool(name="gp", bufs=1))      # gelu acts
            sp_ = ctx.enter_context(tc.tile_pool(name="sp", bufs=1))     # small stat rows
            tp = ctx.enter_context(tc.tile_pool(name="tp", bufs=1))      # [128,S] temps
            mmp = ctx.enter_context(tc.tile_pool(name="mmp", bufs=4, space="PSUM"))

            # ---- weights / constants (resident); weight DMAs deferred until
            # after the first x-shard load so compute starts immediately ----
            wq_s = wp.tile([128, NC2, 2, D], f8, name="wq_s")
            wk_s = wp.tile([128, NC2, 2, D], f8, name="wk_s")
            wv_s = wp.tile([128, NC2, 2, D], f8, name="wv_s")
            wo_s = wp.tile([128, NC2, 2, D], f8, name="wo_s")
            w1_s = wp.tile([128, NC2, 2, F], f8, name="w1_s")
            w2_s = wp.tile([128, NF2, 2, D], f8, name="w2_s")

            def emit_load_weights():
                for t_, d_ in ((wq_s, wq_d), (wk_s, wk_d), (wv_s, wv_d),
                               (wo_s, wo_d), (w1_s, w1_d), (w2_s, w2_d)):
                    nc.sync.dma_start(out=t_[:, :, :, :],
                                      in_=d_.ap().rearrange("c j p n -> p c j n"))
            bqs = wp.tile([128, NCD], f32, name="bqs")
            nc.sync.dma_start(out=bqs[:, :], in_=bq_d.ap().rearrange("c p -> p c"))
            bks = wp.tile([128, NCD], f32, name="bks")
            nc.sync.dma_start(out=bks[:, :], in_=bk_d.ap().rearrange("c p -> p c"))
            bvs = wp.tile([1, D], bf16, name="bvs")
            nc.sync.dma_start(out=bvs[:, :], in_=bv_d[:, :])
            bos = wp.tile([128, NCD], f32, name="bos")
            nc.sync.dma_start(out=bos[:, :], in_=bo_d.ap().rearrange("c p -> p c"))
            wosum_s = wp.tile([128, NC2, 2, 32], f8, name="wosum_s")
            nc.sync.dma_start(out=wosum_s[:, :, :, :],
                              in_=wosum_d.ap().rearrange("c j p m -> p c j m"))
            borow = wp.tile([1, D], bf16, name="borow")
            nc.sync.dma_start(out=borow[:, :], in_=borow_d[:, :])
            b2row = wp.tile([1, D], bf16, name="b2row")
            nc.sync.dma_start(out=b2row[:, :], in_=b2row_d[:, :])
            b1s = wp.tile([128, NCF], f32, name="b1s")
            nc.sync.dma_start(out=b1s[:, :], in_=b1_d.ap().rearrange("c p -> p c"))
            b2s = wp.tile([128, NCD], f32, name="b2s")
            nc.sync.dma_start(out=b2s[:, :], in_=b2_d.ap().rearrange("c p -> p c"))
            ones1 = wp.tile([1, 128], bf16, name="ones1")
            nc.vector.memset(ones1[:, :], 1.0)
            # 32-wide ones (pair layout): DoubleRow needs pair step % 16 == 0 and
            # a >=32-partition matmul destination
            ones8 = wp.tile([128, 2, 32], f8, name="ones8")
            nc.vector.memset(ones8[:, :, :], 1.0)
            eps_s = wp.tile([1, 1], f32, name="eps_s")
            nc.vector.memset(eps_s[:, :], EPS)
            onesS = wp.tile([1, S], bf16, name="onesS")
            nc.vector.memset(onesS[:, :], 1.0)
            ones128f = wp.tile([128, 1], f32, name="ones128f")
            nc.vector.memset(ones128f[:, :], 1.0)
            # bias-of-v broadcast tile [128, D] bf16 (built once via rank-1 matmul)
            bvt = wp.tile([128, D], bf16, name="bvt")
            bvp = mmp.tile([128, D], f32, name="bvp", tag="mm", padded_shape=[128, 1024])
            for (s0, sn) in DSPL:
                nc.tensor.matmul(bvp[:, s0:s0 + sn], ones1[0:1, :], bvs[0:1, s0:s0 + sn],
                                 start=True, stop=True)
            nc.vector.tensor_scalar_mul(bvt[:, :], bvp[:, 0:D], 1.0)
            # scalar sum(bo)/D for the LN2 mean (runtime value)
            bop = mmp.tile([128, 64], f32, name="bop", tag="mm", padded_shape=[128, 1024])
            nc.tensor.matmul(bop[0:1, 0:NCD], ones128f[:, :], bos[:, :],
                             start=True, stop=True)
            bosum = wp.tile([1, 1], f32, name="bosum")
            nc.vector.tensor_reduce(bosum[0:1, 0:1], bop[0:1, 0:NCD],
                                    mybir.AxisListType.X, ALU.add)
            nc.vector.tensor_scalar_mul(bosum[0:1, 0:1], bosum[0:1, 0:1], 1.0 / D)

            # ---------------- helpers ----------------
            def emit_stats(x8t, sq8t, spt, sqt, c2, first, last):
                """DoubleRow ones-matmul accumulation of per-column sums (spt)
                and sums of squares (sqt), rows 0-31 of separate base-0 PSUM
                tiles (DoubleRow requires dst base partition 0).  DoubleRow for
                the 512-wide split only (FD>=128); plain fp8 for the 65 tail."""
                for (s0, sn) in SSPL:
                    if sn >= 128:
                        nc.tensor.matmul(spt[0:32, s0:s0 + sn], ones8[:, :, :],
                                         x8t[:, c2, :, s0:s0 + sn],
                                         start=first, stop=last, perf_mode=DR)
                        nc.tensor.matmul(sqt[0:32, s0:s0 + sn], ones8[:, :, :],
                                         sq8t[:, c2, :, s0:s0 + sn],
                                         start=first, stop=last, perf_mode=DR)
                    else:
                        for j in range(2):
                            nc.tensor.matmul(spt[0:32, s0:s0 + sn], ones8[:, j, :],
                                             x8t[:, c2, j, s0:s0 + sn],
                                             start=first and j == 0,
                                             stop=last and j == 1)
                            nc.tensor.matmul(sqt[0:32, s0:s0 + sn], ones8[:, j, :],
                                             sq8t[:, c2, j, s0:s0 + sn],
                                             start=first and j == 0,
                                             stop=last and j == 1)

            def emit_chain(spt, sqt, mu_bias=None):
                """LN scalar chain on [1,S] rows; outputs AZ*rstd and -AZ*mu*rstd."""
                mu_s = sp_.tile([1, S], f32, name="mu_s", tag="mu_s", bufs=2)
                if mu_bias is None:
                    nc.vector.tensor_scalar_mul(mu_s[:, :], spt[0:1, :], 1.0 / D)
                else:
                    nc.vector.tensor_scalar(mu_s[:, :], spt[0:1, :], 1.0 / D,
                                            mu_bias, op0=ALU.mult, op1=ALU.add)
                v_s = sp_.tile([1, S], f32, name="v_s", tag="v_s", bufs=2)
                nc.scalar.activation(v_s[:, :], sqt[0:1, :], AF.Copy, scale=1.0 / D)
                nc.vector.tensor_mul(spt[0:1, :], mu_s[:, :], mu_s[:, :])
                nc.vector.tensor_sub(v_s[:, :], v_s[:, :], spt[0:1, :])
                w_s = tp.tile([1, S], f32, name="w_s", tag="w_s", bufs=2)
                nc.scalar.activation(w_s[:, :], v_s[:, :], AF.Sqrt, bias=eps_s[0:1, 0:1])
                nc.vector.reciprocal_approx_fast(v_s[:, :], w_s[:, :])
                rstd_bf = sp_.tile([1, S], bf16, name="rstd_bf", tag="rstdbf", bufs=2)
                nc.scalar.activation(rstd_bf[:, :], v_s[:, :], AF.Copy, scale=AZ)
                nmr_bf = sp_.tile([1, S], bf16, name="nmr_bf", tag="nmrbf", bufs=2)
                nc.vector.scalar_tensor_tensor(nmr_bf[:, :], mu_s[:, :], -AZ, v_s[:, :],
                                               op0=ALU.mult, op1=ALU.mult)
                return rstd_bf, nmr_bf

            def emit_bcast(row_bf):
                """Broadcast a [1,S] bf16 row across 128 partitions via rank-1 matmul."""
                bc = mmp.tile([128, S], f32, name="bc", tag="mm", padded_shape=[128, 1024])
                for (s0, sn) in SSPL:
                    nc.tensor.matmul(bc[:, s0:s0 + sn], ones1[0:1, :], row_bf[0:1, s0:s0 + sn],
                                     start=True, stop=True)
                return bc

            def emit_znorm(src, rstd_bc, nmr_bc, z):
                """z[:,c2,j,:] = fp8(src[:,c,:] * (AZ*rstd) + (-AZ*mu*rstd))."""
                for c in range(NCD):
                    c2, j = divmod(c, 2)
                    zt = tp.tile([128, S], bf16, name="zt", tag="zt", bufs=2)
                    nc.vector.tensor_mul(zt[:, :], src[:, c, :], rstd_bc[:, 0:S])
                    nc.vector.tensor_add(z[:, c2, j, 0:S], zt[:, :], nmr_bc[:, 0:S])

            # ---------------- phase emitters ----------------
            xt_tiles = [None] * bpc
            x8_tiles = [None] * bpc
            sq8_tiles = [None] * bpc
            z1_tiles = [None] * bpc
            z2_tiles = [None] * bpc
            x2_tiles = [None] * bpc
            ch1 = [None] * bpc
            ch2 = [None] * bpc

            def emit_load_x(b):
                x8t = zp.tile([128, NC2, 2, SP], f8, name="x8t", tag="x8t", bufs=2)
                nc.sync.dma_start(out=x8t[:, :, :, 0:S],
                                  in_=x8_d[b].rearrange("c j p s -> p c j s"))
                sq8t = zp.tile([128, NC2, 2, SP], f8, name="sq8t", tag="sq8t", bufs=2)
                nc.sync.dma_start(out=sq8t[:, :, :, 0:S],
                                  in_=sq8_d[b].rearrange("c j p s -> p c j s"))
                xt = rp.tile([128, NCD, S], f32, name="xt", tag="xt", bufs=2)
                for c in range(NCD):
                    nc.sync.dma_start(out=xt[:, c, :],
                                      in_=xT_d[b, c * 128:(c + 1) * 128, :])
                x8_tiles[b], sq8_tiles[b], xt_tiles[b] = x8t, sq8t, xt

            def emit_stats1(b):
                spt = mmp.tile([128, S], f32, name="spt", tag="mm", padded_shape=[128, 1024])
                sqt = mmp.tile([128, S], f32, name="sqt", tag="mm", padded_shape=[128, 1024])
                for c2 in range(NC2):
                    emit_stats(x8_tiles[b], sq8_tiles[b], spt, sqt, c2,
                               c2 == 0, c2 == NC2 - 1)
                ch1[b] = emit_chain(spt, sqt)

            def emit_zfinish1(b):
                rstd_bf, nmr_bf = ch1[b]
                rbc = emit_bcast(rstd_bf)
                nbc = emit_bcast(nmr_bf)
                z1 = zp.tile([128, NC2, 2, SP], f8, name="z1", tag="z1", bufs=2)
                emit_znorm(xt_tiles[b], rbc, nbc, z1)
                z1_tiles[b] = z1

            def emit_attention(b, interleave=()):
                """interleave: callables emitted between head-pairs to feed the
                PE while ACT grinds the softmax exps."""
                interleave = list(interleave)
                z1 = z1_tiles[b]
                # --- QKV projections (DoubleRow fp8) ---
                qt = qkp.tile([128, NHP, S], f8, name="qt", tag="qt")
                kt = qkp.tile([128, NHP, S], f8, name="kt", tag="kt")
                for hp in range(NHP):
                    hc = slice(hp * 128, (hp + 1) * 128)
                    qps = mmp.tile([128, S], f32, name="qps", tag="mm", padded_shape=[128, 1024])
                    for (s0, sn) in SSPL:
                        if sn >= 128:
                            for c2 in range(NC2):
                                nc.tensor.matmul(qps[:, s0:s0 + sn], wq_s[:, c2, :, hc],
                                                 z1[:, c2, :, s0:s0 + sn],
                                                 start=(c2 == 0), stop=(c2 == NC2 - 1),
                                                 perf_mode=DR)
                        else:
                            for c in range(NCD):
                                c2, j = divmod(c, 2)
                                nc.tensor.matmul(qps[:, s0:s0 + sn], wq_s[:, c2, j, hc],
                                                 z1[:, c2, j, s0:s0 + sn],
                                                 start=(c == 0), stop=(c == NCD - 1))
                    nc.vector.tensor_scalar(qt[:, hp, :], qps[:, 0:S],
                                            AQ / (AZ * AW), bqs[:, hp:hp + 1],
                                            op0=ALU.mult, op1=ALU.add)
                    kps = mmp.tile([128, S], f32, name="kps", tag="mm", padded_shape=[128, 1024])
                    for (s0, sn) in SSPL:
                        if sn >= 128:
                            for c2 in range(NC2):
                                nc.tensor.matmul(kps[:, s0:s0 + sn], wk_s[:, c2, :, hc],
                                                 z1[:, c2, :, s0:s0 + sn],
                                                 start=(c2 == 0), stop=(c2 == NC2 - 1),
                                                 perf_mode=DR)
                        else:
                            for c in range(NCD):
                                c2, j = divmod(c, 2)
                                nc.tensor.matmul(kps[:, s0:s0 + sn], wk_s[:, c2, j, hc],
                                                 z1[:, c2, j, s0:s0 + sn],
                                                 start=(c == 0), stop=(c == NCD - 1))
                    nc.vector.tensor_scalar(kt[:, hp, :], kps[:, 0:S],
                                            AQ / (AZ * AW), bks[:, hp:hp + 1],
                                            op0=ALU.mult, op1=ALU.add)
                # V in natural layout [t, v]: z1 chunks as stationary, wv moving
                v = qkp.tile([128, len(TCH), D], bf16, name="v", tag="v")
                vsc = qkp.tile([128, len(TCH), D], f8, name="vsc", tag="vsc")
                for it, (t0, tw) in enumerate(TCH):
                    vps = mmp.tile([128, D], f32, name="vps", tag="mm", padded_shape=[128, 1024])
                    for (s0, sn) in DSPL:
                        for c2 in range(NC2):
                            nc.tensor.matmul(vps[0:tw, s0:s0 + sn],
                                             z1[:, c2, :, t0:t0 + tw],
                                             wv_s[:, c2, :, s0:s0 + sn],
                                             start=(c2 == 0), stop=(c2 == NC2 - 1),
                                             perf_mode=DR)
                    nc.vector.tensor_add(v[0:tw, it, :], vps[0:tw, 0:D], bvt[0:tw, :])

                # --- per-head-pair attention ---
                concat = qkp.tile([128, NC2, 2, SP], f8, name="concat", tag="concat")
                for hp in range(NHP):
                    etiles = [None, None]
                    rstiles = [None, None]
                    rectiles = [None, None]
                    for h2 in range(2):
                        hb = h2 * 64
                        e = ep.tile([128, len(TCH), SP], f8, name="e", tag="e")
                        rs = sp_.tile([128, len(TCH)], f32, name="rs", tag="rs", bufs=2)
                        rec = sp_.tile([128, len(TCH)], f32, name="rec", tag="rec", bufs=2)
                        hcol = slice(hp * 128 + hb, hp * 128 + hb + 64)
                        for it, (t0, tw) in enumerate(TCH):
                            stps = mmp.tile([128, S], f32, name="stps", tag="mm",
                                            padded_shape=[128, 1024])
                            for (s0, sn) in SSPL:
                                nc.tensor.matmul(stps[0:tw, s0:s0 + sn],
                                                 kt[hb:hb + 64, hp, t0:t0 + tw],
                                                 qt[hb:hb + 64, hp, s0:s0 + sn],
                                                 start=True, stop=True)
                            nc.scalar.activation(e[0:tw, it, 0:S], stps[0:tw, 0:S], AF.Exp,
                                                 bias=0.0,
                                                 scale=float(1.0 / (np.sqrt(DH) * AQ * AQ)),
                                                 accum_out=rs[0:tw, it:it + 1])
                            # per-chunk reciprocal + v scaling right behind each
                            # exp: vsc(it) is ready ~2 exps after its accum
                            nc.vector.reciprocal(rec[0:tw, it:it + 1],
                                                 rs[0:tw, it:it + 1])
                            nc.vector.tensor_scalar(vsc[0:tw, it, hcol],
                                                    v[0:tw, it, hcol],
                                                    rec[0:tw, it:it + 1],
                                                    AV / (AZ * AW),
                                                    op0=ALU.mult, op1=ALU.mult)
                        etiles[h2] = e
                        rstiles[h2] = rs
                    # PE work between the exps and attnV so the rec/vsc DVE
                    # latency never stalls the attnV weight loads
                    if interleave:
                        interleave.pop(0)()
                    ap_tiles = [mmp.tile([128, S], f32, name="ap_ps", tag="mm",
                                         padded_shape=[128, 1024]) for _ in range(2)]
                    for h2 in range(2):
                        hb = h2 * 64
                        ap_ps = ap_tiles[h2]
                        hcol = slice(hp * 128 + hb, hp * 128 + hb + 64)
                        e = etiles[h2]
                        for (s0, sn) in SSPL:
                            if sn >= 128:
                                nc.tensor.matmul(ap_ps[0:64, s0:s0 + sn],
                                                 vsc[0:128, 0:2, hcol],
                                                 e[0:128, 0:2, s0:s0 + sn],
                                                 start=True, stop=False, perf_mode=DR)
                                nc.tensor.matmul(ap_ps[0:64, s0:s0 + sn],
                                                 vsc[0:128, 2:4, hcol],
                                                 e[0:128, 2:4, s0:s0 + sn],
                                                 start=False, stop=False, perf_mode=DR)
                                lt0, ltw = TCH[-1]
                                nc.tensor.matmul(ap_ps[0:64, s0:s0 + sn],
                                                 vsc[0:ltw, 4, hcol],
                                                 e[0:ltw, 4, s0:s0 + sn],
                                                 start=False, stop=True)
                            else:
                                for it, (t0, tw) in enumerate(TCH):
                                    nc.tensor.matmul(ap_ps[0:64, s0:s0 + sn],
                                                     vsc[0:tw, it, hcol],
                                                     e[0:tw, it, s0:s0 + sn],
                                                     start=(it == 0),
                                                     stop=(it == len(TCH) - 1))
                    c2h, jh = divmod(hp, 2)
                    nc.vector.tensor_scalar_mul(concat[0:64, c2h, jh, 0:S],
                                                ap_tiles[0][0:64, 0:S], ACAT / AV)
                    nc.vector.tensor_scalar_mul(concat[64:128, c2h, jh, 0:S],
                                                ap_tiles[1][0:64, 0:S], ACAT / AV)

                # --- output projection + residual + fused LN2 stats.
                # Sums: sum_d(x2) = sum_d(x) [from x8] + colsum(Wo).concat + sum(bo)
                # (the bo constant enters via the LN2 chain mu bias).
                # Sumsq: ACT Square of each x2 chunk (fp8) + plain fp8 matmuls. ---
                x2 = rp.tile([128, NCD, S], f32, name="x2", tag="x2", bufs=2)
                sq28 = zp.tile([128, NCD, SP], f8, name="sq28", tag="sq28", bufs=1)
                spt = mmp.tile([128, S], f32, name="spt2", tag="mm", padded_shape=[128, 1024])
                sqt = mmp.tile([128, S], f32, name="sqt2", tag="mm", padded_shape=[128, 1024])
                xt = xt_tiles[b]
                x8t = x8_tiles[b]
                for (s0, sn) in SSPL:
                    for c2 in range(NC2):
                        if sn >= 128:
                            nc.tensor.matmul(spt[0:32, s0:s0 + sn], ones8[:, :, :],
                                             x8t[:, c2, :, s0:s0 + sn],
                                             start=(c2 == 0), stop=False, perf_mode=DR)
                            nc.tensor.matmul(spt[0:32, s0:s0 + sn], wosum_s[:, c2, :, :],
                                             concat[:, c2, :, s0:s0 + sn],
                                             start=False, stop=(c2 == NC2 - 1),
                                             perf_mode=DR)
                        else:
                            for j in range(2):
                                nc.tensor.matmul(spt[0:32, s0:s0 + sn], ones8[:, j, :],
                                                 x8t[:, c2, j, s0:s0 + sn],
                                                 start=(c2 == 0 and j == 0), stop=False)
                                nc.tensor.matmul(spt[0:32, s0:s0 + sn],
                                                 wosum_s[:, c2, j, :],
                                                 concat[:, c2, j, s0:s0 + sn],
                                                 start=False,
                                                 stop=(c2 == NC2 - 1 and j == 1))
                for ec in range(NCD):
                    wops = mmp.tile([128, S], f32, name="wops", tag="mm",
                                    padded_shape=[128, 1024])
                    for (s0, sn) in SSPL:
                        if sn >= 128:
                            for cc in range(NC2):
                                nc.tensor.matmul(wops[:, s0:s0 + sn],
                                                 wo_s[:, cc, :, ec * 128:(ec + 1) * 128],
                                                 concat[:, cc, :, s0:s0 + sn],
                                                 start=(cc == 0), stop=False,
                                                 perf_mode=DR)
                        else:
                            for cc in range(NCD):
                                c2c, jc = divmod(cc, 2)
                                nc.tensor.matmul(wops[:, s0:s0 + sn],
                                                 wo_s[:, c2c, jc, ec * 128:(ec + 1) * 128],
                                                 concat[:, c2c, jc, s0:s0 + sn],
                                                 start=(cc == 0), stop=False)
                        # bias bo via bf16 rank-1 (keeps the SBUF copy a single STT op)
                        nc.tensor.matmul(wops[:, s0:s0 + sn],
                                         borow[0:1, ec * 128:(ec + 1) * 128],
                                         onesS[0:1, s0:s0 + sn],
                                         start=False, stop=True)
                    nc.vector.scalar_tensor_tensor(x2[:, ec, :], wops[:, 0:S],
                                                   1.0 / (ACAT * AW), xt[:, ec, :],
                                                   op0=ALU.mult, op1=ALU.add)
                    nc.vector.tensor_mul(sq28[:, ec, 0:S], x2[:, ec, :], x2[:, ec, :])
                    for (s0, sn) in SSPL:
                        nc.tensor.matmul(sqt[0:32, s0:s0 + sn], ones8[:, 0, :],
                                         sq28[:, ec, s0:s0 + sn],
                                         start=(ec == 0), stop=(ec == NCD - 1))
                x2_tiles[b] = x2
                return spt, sqt

            def emit_stats2(b, spts):
                ch2[b] = emit_chain(*spts, mu_bias=bosum[0:1, 0:1])

            def emit_zfinish2(b):
                rstd_bf, nmr_bf = ch2[b]
                rbc = emit_bcast(rstd_bf)
                nbc = emit_bcast(nmr_bf)
                z2 = zp.tile([128, NC2, 2, SP], f8, name="z2", tag="z2", bufs=1)
                emit_znorm(x2_tiles[b], rbc, nbc, z2)
                z2_tiles[b] = z2

            def emit_fc1(b, g, lo, hi):
                z2 = z2_tiles[b]
                for fc in range(lo, hi):
                    fps = mmp.tile([128, S], f32, name="fps", tag="mm",
                                   padded_shape=[128, 1024])
                    for (s0, sn) in SSPL:
                        if sn >= 128:
                            for c2 in range(NC2):
                                nc.tensor.matmul(fps[:, s0:s0 + sn],
                                                 w1_s[:, c2, :, fc * 128:(fc + 1) * 128],
                                                 z2[:, c2, :, s0:s0 + sn],
                                                 start=(c2 == 0), stop=(c2 == NC2 - 1),
                                                 perf_mode=DR)
                        else:
                            for c in range(NCD):
                                c2, j = divmod(c, 2)
                                nc.tensor.matmul(fps[:, s0:s0 + sn],
                                                 w1_s[:, c2, j, fc * 128:(fc + 1) * 128],
                                                 z2[:, c2, j, s0:s0 + sn],
                                                 start=(c == 0), stop=(c == NCD - 1))
                    cf, jf = divmod(fc, 2)
                    nc.scalar.activation(g[:, cf, jf, 0:S], fps[:, 0:S], GELU,
                                         bias=b1s[:, fc:fc + 1], scale=1.0 / (AZ * AW))

            def emit_fc2_chunk(b, g, ec):
                x2 = x2_tiles[b]
                p2 = mmp.tile([128, S], f32, name="p2", tag="mm", padded_shape=[128, 1024])
                for (s0, sn) in SSPL:
                    if sn >= 128:
                        for cf in range(NF2):
                            nc.tensor.matmul(p2[:, s0:s0 + sn],
                                             w2_s[:, cf, :, ec * 128:(ec + 1) * 128],
                                             g[:, cf, :, s0:s0 + sn],
                                             start=(cf == 0), stop=False,
                                             perf_mode=DR)
                    else:
                        for cf in range(NCF):
                            c2f, jf = divmod(cf, 2)
                            nc.tensor.matmul(p2[:, s0:s0 + sn],
                                             w2_s[:, c2f, jf, ec * 128:(ec + 1) * 128],
                                             g[:, c2f, jf, s0:s0 + sn],
                                             start=(cf == 0), stop=False)
                    nc.tensor.matmul(p2[:, s0:s0 + sn],
                                     b2row[0:1, ec * 128:(ec + 1) * 128],
                                     onesS[0:1, s0:s0 + sn],
                                     start=False, stop=True)
                o1 = tp.tile([128, S], f32, name="o1", tag="o1", bufs=2)
                nc.vector.scalar_tensor_tensor(o1[:, :], p2[:, 0:S], 1.0 / AW,
                                               x2[:, ec, :],
                                               op0=ALU.mult, op1=ALU.add)
                nc.sync.dma_start(out=outT_d[b, ec * 128:(ec + 1) * 128, :],
                                  in_=o1[:, :])

            # ---------------- emission schedule ----------------
            # Two-deep pipeline: FC2(b) is emitted after attention(b+1) so
            # batch b+1's LN2 chain hides under FC2(b)'s matmuls; chain1(b+1)
            # hides under FC1(b); LN2 stats matmuls are fused into Wo.
            emit_load_x(0)
            emit_load_weights()
            emit_stats1(0)          # chain1(0) — exposed at startup only
            emit_zfinish1(0)
            if bpc > 1:
                emit_load_x(1)      # x(b+1) DMAs hide under attention(b)
            spt2 = emit_attention(0)
            emit_stats2(0, spt2)    # chain2(0) — exposed once (no prior FC2)
            for b in range(bpc):
                emit_zfinish2(b)
                g = gp.tile([128, NF2, 2, SP], f8, name="g", tag="g")
                emit_fc1(b, g, 0, 3)
                if b + 1 < bpc:
                    emit_stats1(b + 1)      # chain1(b+1): ACT ops queue ahead
                                            # of most gelus (strict FIFO)
                emit_fc1(b, g, 3, NCF // 2)
                if b + 1 < bpc:
                    emit_zfinish1(b + 1)    # z1(b+1) DVE runs alongside
                emit_fc1(b, g, NCF // 2, NCF)
                if b + 1 < bpc:
                    if b + 2 < bpc:
                        emit_load_x(b + 2)  # prefetch two stages ahead
                    # FC2(b) chunks 0-4 feed the PE between attention(b+1)
                    # head-pairs (ACT-bound); chunk 5 lands after Wo so
                    # chain2(b+1) hides under it.
                    il = [(lambda ec=ec: emit_fc2_chunk(b, g, ec)) for ec in range(5)]
                    spt2 = emit_attention(b + 1, interleave=il)
                    emit_stats2(b + 1, spt2)
                    emit_fc2_chunk(b, g, 5)
                else:
                    for ec in range(NCD):
                        emit_fc2_chunk(b, g, ec)
    nc.finalize()
    return nc


def _get_nc(gelu_kind: str = "gelu", bpc: int = BPC):
    key = (gelu_kind, bpc)
    if key not in _NC_CACHE:
        _NC_CACHE[key] = _build_nc(gelu_kind, bpc)
    return _NC_CACHE[key]


def _pair_rows(w):
    """[D_in, N] -> [NC_in/256, 2, 128, N] fp8 pair-chunk layout."""
    d_in, n = w.shape
    return np.ascontiguousarray(w.reshape(d_in // 256, 2, 128, n))


def _prep_inputs(inputs):
    """Host-side prep: fp8 weight quantization (scaled by AW), pair layouts,
    folded LN affines, fp8 x / x^2 stats inputs.  Returns per-core input maps."""
    f8 = ml_dtypes.float8_e4m3
    bf16 = ml_dtypes.bfloat16
    f32 = np.float32
    g1 = np.asarray(inputs["ln1_g"], f32)
    b1l = np.asarray(inputs["ln1_b"], f32)
    g2 = np.asarray(inputs["ln2_g"], f32)
    b2l = np.asarray(inputs["ln2_b"], f32)

    def flat(Wx):  # [H, D, DH] -> [D, H*DH]
        return np.ascontiguousarray(np.transpose(np.asarray(Wx, f32), (1, 0, 2)).reshape(D, D))

    wq_f, wk_f, wv_f = flat(inputs["Wq"]), flat(inputs["Wk"]), flat(inputs["Wv"])
    W1 = np.asarray(inputs["W1"], f32)
    W2 = np.asarray(inputs["W2"], f32)
    Wo = np.asarray(inputs["Wo"], f32)
    bo = np.asarray(inputs["bo"], f32)
    b2b = np.asarray(inputs["b2"], f32)
    # column sums of the *quantized* Wo (for the analytic LN2 mean), scaled 1/ACAT
    wo_q = (AW * Wo).astype(f8).astype(f32) / AW
    wosum = (wo_q.sum(axis=1) / ACAT).astype(f8)          # [D]
    wosum_rep = np.broadcast_to(
        wosum.reshape(NC2, 2, 128, 1), (NC2, 2, 128, 32))
    w = {
        "wq": _pair_rows((AW * g1[:, None] * wq_f).astype(f8)),
        "wk": _pair_rows((AW * g1[:, None] * wk_f).astype(f8)),
        "wv": _pair_rows((AW * g1[:, None] * wv_f).astype(f8)),
        "wo": _pair_rows((AW * Wo).astype(f8)),
        "w1": _pair_rows((AW * g2[:, None] * W1).astype(f8)),
        "w2": _pair_rows((AW * W2).astype(f8)),
        # qt = psum*(AQ/(AZ*AW)) + AQ*bq_fold
        "bq": (AQ * (b1l @ wq_f + np.asarray(inputs["bq"], f32).reshape(-1))).reshape(NCD, 128).astype(f32),
        "bk": (AQ * (b1l @ wk_f + np.asarray(inputs["bk"], f32).reshape(-1))).reshape(NCD, 128).astype(f32),
        # v_sb = psum + (AZ*AW)*bv_fold
        "bv": (AZ * AW * (b1l @ wv_f + np.asarray(inputs["bv"], f32).reshape(-1))).reshape(1, D).astype(bf16),
        "bo": bo.reshape(NCD, 128).copy(),
        "b1": (b2l @ W1 + np.asarray(inputs["b1"], f32)).reshape(NCF, 128).astype(f32),
        "b2": b2b.reshape(NCD, 128).copy(),
        "wosum": np.ascontiguousarray(wosum_rep),
        "borow": (ACAT * AW * bo).reshape(1, D).astype(bf16),
        "b2row": (AW * b2b).reshape(1, D).astype(bf16),
    }
    x = np.asarray(inputs["x"], np.float32)
    # shard over batch, transpose to [b, D, S] per core
    xT = np.ascontiguousarray(
        x.reshape(NCORES, BPC, S, D).swapaxes(2, 3))  # [8, BPC, D, S]
    x8 = xT.astype(f8)                                # [8, BPC, D, S] fp8
    x8f = x8.astype(f32)
    sq8 = (x8f * x8f).astype(f8)
    x8 = x8.reshape(NCORES, BPC, NC2, 2, 128, S)
    sq8 = sq8.reshape(NCORES, BPC, NC2, 2, 128, S)
    return [dict(w, xT=xT[i], x8=x8[i], sq8=sq8[i]) for i in range(NCORES)]


def kernel(**inputs) -> np.ndarray:
    from concourse.bass_utils import run_bass_kernel_spmd

    nc = _get_nc("gelu", BPC)
    in_maps = _prep_inputs(inputs)
    res = run_bass_kernel_spmd(nc, in_maps, core_ids=list(range(NCORES)))
    outs = [res.results[i]["outT"] for i in range(NCORES)]   # each [BPC, D, S]
    out = np.stack(outs, 0).swapaxes(2, 3).reshape(B, S, D)
    return np.ascontiguousarray(out.astype(np.float32))
